# revision 9
# baseline (speedup 1.0000x reference)
"""Trainium2 Bass kernel for nn_CombinedTargetIOULoss (B=64, K=17, H=W=64).

Data-parallel over batch: 8 cores x 8 batches. Per core the six
component planes (o/t x ox/oy/hm) are DMA'd f32 via the sync-engine
hardware DGE with 4KB contiguous descriptors into one SBUF allocation
with partition = local_batch*16 + k for joints k<16, pixels in the free
dim. Joint k=16 rides in a sidecar tile with partition =
local_batch*16 + (px>>8).

Per pixel-quarter the box algebra runs element-wise, with x/y axes
paired into single ops via strided views of the plane allocation:
  iw = relu(min(p,g)) + relu(-max(p,g))   (custom DVE op, both axes)
  cw = relu(max(p,g)) + relu(-min(p,g))   (custom DVE op)
  inter = iw*ih, ac = cw*ch, s = |p*q| + |g*h|, ue = s - inter
  q1 = inter/(ue+eps), q2 = ue/(ac+eps), dsq = (hm_o - hm_t)^2
Reciprocals run on the scalar (ACT) engine (f32 out + f16 copy), the
pq/gh pair and hm diff on gpsimd, everything else f16 on DVE (2x).
Per-(b,k) pixel sums come free from accum_out on ACT Copy/Square ops.
The host combines the [128,16] partials into the scalar loss.
"""

import sys

sys.path.insert(0, "/opt/trn_rl_repo")

import numpy as np

import concourse.bass as bass
import concourse.dve_ops as dve_ops
from concourse import mybir
from concourse.alu_op_type import AluOpType as Alu
from concourse.bass_utils import run_bass_kernel_spmd
from concourse.dve_spec import Spec, Src0, Src1, Zero, lower, maxx, minn, relu
from concourse.dve_uop import DveOpSpec

F32 = mybir.dt.float32
F16 = mybir.dt.float16
AF = mybir.ActivationFunctionType

EPS = 2e-4
B, K, H, W = 64, 17, 64, 64
P = H * W              # 4096 pixels
N_CORES = 8
B_LOC = B // N_CORES   # 8 batches per core
KM = 16                # joints in the main tiles; k=16 is the sidecar
QP = P // 4            # pixels per quarter (1024)
SP = P // 16           # sidecar pixels per partition (256)

ND, NA, NG = 11, 7, 0  # ops per chain-step per engine
CHUNKS = [(0, 512), (512, 512), (1024, 1024), (2048, 1024), (3072, 1024)]


def _register(name, body, ref):
    """Register a custom DVE op (idempotent across re-imports)."""
    for op in dve_ops.OPS:
        if op.name == name:
            return op
    op = dve_ops.DveOp(name, Spec(body=body, reference=ref), subdim=False,
                       uops_sha={})
    shas = {}
    for ver in ("v3", "v4"):
        s = DveOpSpec(name=name, opcode=0, uops=lower(op.spec, ver=ver),
                      rd1_en=True)
        shas[ver] = s.sha(ver)
    object.__setattr__(op, "uops_sha", shas)
    dve_ops.OPS.append(op)
    dve_ops.CUSTOM_DVE_SPECS[name] = op.spec
    dve_ops._SUB_OPCODE_FOR_NAME[name] = (
        dve_ops._CUSTOM_DVE_ROW_BASE + len(dve_ops.OPS) - 1
    )
    assert dve_ops._SUB_OPCODE_FOR_NAME[name] < 0x20
    return op


def _ref_iw(in0, in1, s0, s1, imm2):
    mn = np.minimum(in0, in1)
    mx = np.maximum(in0, in1)
    return np.maximum(mn, 0) + np.maximum(-mx, 0)


def _ref_cw(in0, in1, s0, s1, imm2):
    mn = np.minimum(in0, in1)
    mx = np.maximum(in0, in1)
    return np.maximum(mx, 0) + np.maximum(-mn, 0)


_mn = minn(Src0, Src1)
_mx = maxx(Src0, Src1)
IW_OP = _register("IW_ANT", relu(_mn) + relu(Zero - _mx), _ref_iw)
CW_OP = _register("CW_ANT", relu(_mx) + relu(Zero - _mn), _ref_cw)


class _Waiter:
    """Dedupe monotone standalone waits per (engine, sem)."""

    def __init__(self):
        self.seen = {}

    def wait(self, eng, sem, val):
        if val <= 0:
            return
        key = (id(eng), sem.name if hasattr(sem, "name") else id(sem))
        if self.seen.get(key, -1) >= val:
            return
        self.seen[key] = val
        eng.wait_ge(sem, val)


def build_nc():
    nc = bass.Bass()
    o_ext = nc.declare_dram_parameter("output", [B_LOC, 3 * K, H, W], F32,
                                      isOutput=False)
    t_ext = nc.declare_dram_parameter("target", [B_LOC, 3 * K, H, W], F32,
                                      isOutput=False)
    p_ext = nc.declare_dram_parameter("partials", [128, 24], F32, isOutput=True)

    sb = lambda name, shape, dt: nc.alloc_sbuf_tensor(name, shape, dt).ap()

    # planes: one alloc, free order (tensor o/t, comp x/y/h, px)
    mpl = sb("mpl", [128, 6 * P], F16)
    spl = sb("spl", [128, 6 * SP], F16)
    mpl4 = mpl.rearrange("p (t c px) -> p t c px", t=2, c=3, px=P)
    spl4 = spl.rearrange("p (t c px) -> p t c px", t=2, c=3, px=SP)

    def mk_mids(pre, n):
        m = {}
        for nm, width, dt, bufs in (
            ("iwih", 2 * n, F16, 1), ("cwch", 2 * n, F16, 1),
            ("pqgh", 2 * n, F16, 2), ("t12", 2 * n, F16, 2),
            ("d", n, F16, 2), ("inter", n, F16, 1), ("ac", n, F16, 2),
            ("s", n, F16, 1), ("ue", n, F16, 2),
            ("rcu32", n, F32, 1), ("rcc32", n, F32, 1),
            ("rcu16", n, F16, 2), ("rcc16", n, F16, 2),
            ("q1", n, F16, 1), ("q2", n, F16, 1), ("q12", n, F16, 2),
            ("ascr", n, F16, 1), ("vscr", n, F16, 1),
        ):
            m[nm] = [sb(f"{pre}{nm}{i}", [128, width], dt) for i in range(bufs)]
        return m

    mm = mk_mids("m_", QP)
    sm = mk_mids("s_", SP)
    osb = sb("osb", [128, 24], F32)
    dmy = sb("dmy", [128, 4], F32)

    dma_side = nc.alloc_semaphore("dma_side")
    dma_q = [nc.alloc_semaphore(f"dma_q{q}") for q in range(len(CHUNKS))]
    dma_xy = [nc.alloc_semaphore(f"dma_xy{q}") for q in range(len(CHUNKS))]
    dma_out = nc.alloc_semaphore("dma_out")
    act_c = nc.alloc_semaphore("act_c")
    dve_c = nc.alloc_semaphore("dve_c")
    gp_c = nc.alloc_semaphore("gp_c")
    init_c = nc.alloc_semaphore("init_c")
    wt = _Waiter()

    # DRAM component-plane views: [b, k, c, px]
    o_v = o_ext.rearrange("b (k c) hx hy -> b k c (hx hy)", k=K, c=3)
    t_v = t_ext.rearrange("b (k c) hx hy -> b k c (hx hy)", k=K, c=3)

    # --- init + ACT table warmup (before any gating) ---
    nc.gpsimd.memset(dmy[:], 0.0).then_inc(init_c, 1)
    nc.gpsimd.memset(osb[:], 0.0).then_inc(init_c, 1)
    nc.vector.wait_ge(init_c, 2)
    nc.scalar.wait_ge(init_c, 2)
    nc.scalar.activation(dmy[:, 0:1], dmy[:, 3:4], AF.Abs)
    nc.scalar.activation(dmy[:, 1:2], dmy[:, 3:4], AF.Square)
    nc.scalar.add_instruction(
        mybir.InstActivation(
            name=nc.get_next_instruction_name(),
            func=AF.Reciprocal,
            ins=[nc.scalar.lower_ap(dmy[:, 3:4]),
                 mybir.ImmediateValue(dtype=F32, value=1.0),
                 mybir.ImmediateValue(dtype=F32, value=1.0),
                 mybir.ImmediateValue(dtype=F32, value=0.0)],
            outs=[nc.scalar.lower_ap(dmy[:, 2:3])],
        )
    )

    # --- DMA (gpsimd SWDGE, f32->f16 cast): geometric px-chunks, side last ---
    for ci2, (p0, pn) in enumerate(CHUNKS):
        # x/y planes first (own sem, gate 64); hm planes after (gate 32)
        for ti, ci in ((0, 1), (0, 2), (1, 1), (1, 2), (0, 0), (1, 0)):
            src = o_v if ti == 0 else t_v
            sem = dma_xy[ci2] if ci else dma_q[ci2]
            nc.gpsimd.dma_start(
                out=mpl4[:, ti, ci, p0:p0 + pn],
                in_=src[:, 0:KM, ci, p0:p0 + pn],
            ).then_inc(sem, 16)
    for ti, src in ((0, o_v), (1, t_v)):
        for ci in range(3):
            # sidecar: k=16, partition = b*16 + (px>>8)
            nc.gpsimd.dma_start(
                out=spl4[:, ti, ci],
                in_=src[:, KM, ci].rearrange("b (ph pl) -> b ph pl",
                                             ph=16, pl=SP),
            ).then_inc(dma_side, 16)

    def act_recip(out, in_, bias):
        return nc.scalar.add_instruction(
            mybir.InstActivation(
                name=nc.get_next_instruction_name(),
                func=AF.Reciprocal,
                ins=[nc.scalar.lower_ap(in_),
                     mybir.ImmediateValue(dtype=F32, value=bias),
                     mybir.ImmediateValue(dtype=F32, value=1.0),
                     mybir.ImmediateValue(dtype=F32, value=0.0)],
                outs=[nc.scalar.lower_ap(out)],
            )
        )

    def chain(step, pl4, n, m, col_q1, col_q2, col_d, gate_sem):
        """One chain-step (side=step0 on [128,SP]; quarter q=step q+1 on a
        QP slice). pl4: [p, t, c, px-window] plane view for this step."""
        d0, a0, g0 = ND * step, NA * step, NG * step
        dp, ap, gp = ND * (step - 2), NA * (step - 2), NG * (step - 2)
        par = step % 2

        def mb(nm):
            bufs = m[nm]
            t = bufs[par % len(bufs)]
            if nm in ("iwih", "cwch", "pqgh", "t12"):
                return t[:]
            return t[:, 0:n]

        o_xy = pl4[:, 0, 1:3]
        t_xy = pl4[:, 1, 1:3]
        x_ot = pl4[:, :, 1]
        y_ot = pl4[:, :, 2]
        o_h = pl4[:, 0, 0]
        t_h = pl4[:, 1, 0]
        iw2 = mb("iwih").rearrange("p (c px) -> p c px", c=2)[:, :, 0:n]
        cw2 = mb("cwch").rearrange("p (c px) -> p c px", c=2)[:, :, 0:n]

        x_ot = pl4[:, :, 1]
        y_ot = pl4[:, :, 2]
        t122 = mb("t12").rearrange("p (c px) -> p c px", c=2)[:, :, 0:n]
        pqw = mb("pqgh").rearrange("p (c px) -> p c px", c=2)[:, :, 0:n]

        # DVE (x/y planes arrive first: front gates on xy sem, d on hm sem)
        if m is mm:
            wt.wait(nc.vector, dma_xy[step], 64)
        else:
            wt.wait(nc.vector, gate_sem, 96)
        nc.vector._custom_dve(IW_OP, out=iw2, in0=o_xy, in1=t_xy).then_inc(dve_c, 1)
        nc.vector._custom_dve(CW_OP, out=cw2, in0=o_xy, in1=t_xy).then_inc(dve_c, 1)
        if step >= 2:
            wt.wait(nc.vector, act_c, ap + 1)       # pqgh read by t12(s-2)
        nc.vector.tensor_tensor(pqw, x_ot, y_ot, Alu.mult).then_inc(dve_c, 1)
        if step >= 2:
            wt.wait(nc.vector, act_c, ap + 7)       # d read by dsq(s-2)
        wt.wait(nc.vector, gate_sem, 32 if m is mm else 96)
        nc.vector.tensor_tensor(mb("d"), o_h, t_h, Alu.subtract).then_inc(dve_c, 1)
        nc.vector.tensor_tensor(mb("inter"), iw2[:, 0], iw2[:, 1], Alu.mult).then_inc(dve_c, 1)
        if step >= 2:
            wt.wait(nc.vector, act_c, ap + 2)       # ac read by rcc32(s-2)
        nc.vector.tensor_tensor(mb("ac"), cw2[:, 0], cw2[:, 1], Alu.mult).then_inc(dve_c, 1)
        wt.wait(nc.vector, act_c, a0 + 1)           # t12 ready
        nc.vector.tensor_tensor(mb("s"), t122[:, 0], t122[:, 1], Alu.add).then_inc(dve_c, 1)
        if step >= 2:
            wt.wait(nc.vector, act_c, ap + 3)       # ue read by rcu32(s-2)
        nc.vector.tensor_tensor(mb("ue"), mb("s"), mb("inter"), Alu.subtract).then_inc(dve_c, 1)
        wt.wait(nc.vector, act_c, a0 + 4)           # rcc16 ready
        nc.vector.tensor_tensor(mb("q2"), mb("ue"), mb("rcc16"), Alu.mult).then_inc(dve_c, 1)
        wt.wait(nc.vector, act_c, a0 + 5)           # rcu16 ready
        nc.vector.tensor_tensor(mb("q1"), mb("inter"), mb("rcu16"), Alu.mult).then_inc(dve_c, 1)
        if step >= 2:
            wt.wait(nc.vector, act_c, ap + 6)       # q12 read by q12red(s-2)
        nc.vector.tensor_tensor(mb("q12"), mb("q1"), mb("q2"), Alu.add).then_inc(dve_c, 1)

        # ACT
        wt.wait(nc.scalar, dve_c, d0 + 3)
        if step >= 2:
            wt.wait(nc.scalar, dve_c, dp + 7)       # t12 read by s(s-2)
        nc.scalar.activation(t122, pqw, AF.Abs).then_inc(act_c, 1)
        wt.wait(nc.scalar, dve_c, d0 + 6)
        act_recip(mb("rcc32"), mb("ac"), EPS).then_inc(act_c, 1)
        wt.wait(nc.scalar, dve_c, d0 + 8)
        act_recip(mb("rcu32"), mb("ue"), EPS).then_inc(act_c, 1)
        if step >= 2:
            wt.wait(nc.scalar, dve_c, dp + 9)       # rcc16 read by q2(s-2)
        nc.scalar.activation(mb("rcc16"), mb("rcc32"), AF.Copy).then_inc(act_c, 1)
        if step >= 2:
            wt.wait(nc.scalar, dve_c, dp + 10)      # rcu16 read by q1(s-2)
        nc.scalar.activation(mb("rcu16"), mb("rcu32"), AF.Copy).then_inc(act_c, 1)
        wt.wait(nc.scalar, dve_c, d0 + 11)
        nc.scalar.activation(mb("ascr"), mb("q12"), AF.Copy,
                             accum_out=col_q1).then_inc(act_c, 1)
        wt.wait(nc.scalar, dve_c, d0 + 4)
        nc.scalar.activation(mb("ascr"), mb("d"), AF.Square,
                             accum_out=col_d).then_inc(act_c, 1)

    # main chunks in order, sidecar last
    NC_ = len(CHUNKS)
    for ci2, (p0, pn) in enumerate(CHUNKS):
        chain(ci2, mpl4[:, :, :, p0:p0 + pn], pn, mm,
              osb[:, ci2:ci2 + 1], osb[:, NC_ + ci2:NC_ + ci2 + 1],
              osb[:, 2 * NC_ + ci2:2 * NC_ + ci2 + 1], dma_q[ci2])
    chain(NC_, spl4, SP, sm,
          osb[:, 3 * NC_:3 * NC_ + 1], osb[:, 3 * NC_ + 1:3 * NC_ + 2],
          osb[:, 3 * NC_ + 2:3 * NC_ + 3], dma_side)

    # epilogue
    wt.wait(nc.sync, dve_c, ND * (len(CHUNKS) + 1))
    wt.wait(nc.sync, act_c, NA * (len(CHUNKS) + 1))
    nc.sync.dma_start(out=p_ext[:], in_=osb[:]).then_inc(dma_out, 16)
    nc.sync.wait_ge(dma_out, 16)

    mybir.codegen_inst_isa_subclasses(nc)
    return nc


_NC = None


def _get_nc():
    global _NC
    if _NC is None:
        _NC = build_nc()
    return _NC


def _combine(parts, target_weights):
    """parts: [8 cores, 128, 16] f32 -> scalar loss (host-side finish)."""
    arr = np.asarray(parts, np.float64).reshape(N_CORES, B_LOC, 16, 24)
    nch = len(CHUNKS)
    sqs = np.zeros((B, K))
    ssd = np.zeros((B, K))
    for i in range(N_CORES):
        for b in range(B_LOC):
            gb = i * B_LOC + b
            rows = arr[i, b]                      # [16 partitions, 24 cols]
            sqs[gb, :KM] = rows[:, 0:2 * nch].sum(1)
            ssd[gb, :KM] = rows[:, 2 * nch:3 * nch].sum(1)
            sqs[gb, KM] = rows[:, 3 * nch].sum() + rows[:, 3 * nch + 1].sum()
            ssd[gb, KM] = rows[:, 3 * nch + 2].sum()

    tw = np.asarray(target_weights, np.float64)
    twnz = (tw != 0).astype(np.float64)
    num = ((2.0 * P - sqs) * twnz).sum(axis=0)
    den = np.maximum((P * twnz).sum(axis=0), 1.0)
    giou_joint = num / den
    mse = 0.5 * (tw**2 * ssd).sum(axis=0) / (B * P)
    return np.float32(np.sum(mse + giou_joint) / K)


def kernel(output, target, target_weights):
    output = np.ascontiguousarray(np.asarray(output), dtype=np.float32)
    target = np.ascontiguousarray(np.asarray(target), dtype=np.float32)
    nc = _get_nc()
    in_maps = [
        {
            "output": output[i * B_LOC:(i + 1) * B_LOC],
            "target": target[i * B_LOC:(i + 1) * B_LOC],
        }
        for i in range(N_CORES)
    ]
    res = run_bass_kernel_spmd(nc, in_maps, list(range(N_CORES)))
    parts = np.stack([res.results[i]["partials"] for i in range(N_CORES)])
    return np.asarray(_combine(parts, target_weights), dtype=np.float32)


# revision 10
# speedup vs baseline: 1.1714x; 1.1714x over previous
"""Trainium2 Bass kernel for nn_CombinedTargetIOULoss (B=64, K=17, H=W=64).

Data-parallel over batch: 8 cores x 8 batches. Per core the six
component planes (o/t x ox/oy/hm) are DMA'd f32 via the sync-engine
hardware DGE with 4KB contiguous descriptors into one SBUF allocation
with partition = local_batch*16 + k for joints k<16, pixels in the free
dim. Joint k=16 rides in a sidecar tile with partition =
local_batch*16 + (px>>8).

Per pixel-quarter the box algebra runs element-wise, with x/y axes
paired into single ops via strided views of the plane allocation:
  iw = relu(min(p,g)) + relu(-max(p,g))   (custom DVE op, both axes)
  cw = relu(max(p,g)) + relu(-min(p,g))   (custom DVE op)
  inter = iw*ih, ac = cw*ch, s = |p*q| + |g*h|, ue = s - inter
  q1 = inter/(ue+eps), q2 = ue/(ac+eps), dsq = (hm_o - hm_t)^2
Reciprocals run on the scalar (ACT) engine (f32 out + f16 copy), the
pq/gh pair and hm diff on gpsimd, everything else f16 on DVE (2x).
Per-(b,k) pixel sums come free from accum_out on ACT Copy/Square ops.
The host combines the [128,16] partials into the scalar loss.
"""

import sys

sys.path.insert(0, "/opt/trn_rl_repo")

import numpy as np

import concourse.bass as bass
import concourse.dve_ops as dve_ops
from concourse import mybir
from concourse.alu_op_type import AluOpType as Alu
from concourse.bass_utils import run_bass_kernel_spmd
from concourse.dve_spec import Spec, Src0, Src1, Zero, lower, maxx, minn, relu
from concourse.dve_uop import DveOpSpec

F32 = mybir.dt.float32
F16 = mybir.dt.float16
AF = mybir.ActivationFunctionType

EPS = 2e-4
B, K, H, W = 64, 17, 64, 64
P = H * W              # 4096 pixels
N_CORES = 8
B_LOC = B // N_CORES   # 8 batches per core
KM = 16                # joints in the main tiles; k=16 is the sidecar
QP = P // 4            # pixels per quarter (1024)
SP = P // 16           # sidecar pixels per partition (256)

ND, NA, NG = 10, 4, 0  # ops per chain-step per engine
CHUNKS = [(0, 512), (512, 512), (1024, 1024), (2048, 1024), (3072, 1024)]


def _register(name, body, ref):
    """Register a custom DVE op (idempotent across re-imports)."""
    for op in dve_ops.OPS:
        if op.name == name:
            return op
    op = dve_ops.DveOp(name, Spec(body=body, reference=ref), subdim=False,
                       uops_sha={})
    shas = {}
    for ver in ("v3", "v4"):
        s = DveOpSpec(name=name, opcode=0, uops=lower(op.spec, ver=ver),
                      rd1_en=True)
        shas[ver] = s.sha(ver)
    object.__setattr__(op, "uops_sha", shas)
    dve_ops.OPS.append(op)
    dve_ops.CUSTOM_DVE_SPECS[name] = op.spec
    dve_ops._SUB_OPCODE_FOR_NAME[name] = (
        dve_ops._CUSTOM_DVE_ROW_BASE + len(dve_ops.OPS) - 1
    )
    assert dve_ops._SUB_OPCODE_FOR_NAME[name] < 0x20
    return op


def _ref_iw(in0, in1, s0, s1, imm2):
    mn = np.minimum(in0, in1)
    mx = np.maximum(in0, in1)
    return np.maximum(mn, 0) + np.maximum(-mx, 0)


def _ref_cw(in0, in1, s0, s1, imm2):
    mn = np.minimum(in0, in1)
    mx = np.maximum(in0, in1)
    return np.maximum(mx, 0) + np.maximum(-mn, 0)


_mn = minn(Src0, Src1)
_mx = maxx(Src0, Src1)
IW_OP = _register("IW_ANT", relu(_mn) + relu(Zero - _mx), _ref_iw)
CW_OP = _register("CW_ANT", relu(_mx) + relu(Zero - _mn), _ref_cw)


class _Waiter:
    """Dedupe monotone standalone waits per (engine, sem)."""

    def __init__(self):
        self.seen = {}

    def wait(self, eng, sem, val):
        if val <= 0:
            return
        key = (id(eng), sem.name if hasattr(sem, "name") else id(sem))
        if self.seen.get(key, -1) >= val:
            return
        self.seen[key] = val
        eng.wait_ge(sem, val)


def build_nc():
    nc = bass.Bass()
    o_ext = nc.declare_dram_parameter("output", [B_LOC, 3 * K, H, W], F32,
                                      isOutput=False)
    t_ext = nc.declare_dram_parameter("target", [B_LOC, 3 * K, H, W], F32,
                                      isOutput=False)
    p_ext = nc.declare_dram_parameter("partials", [128, 24], F32, isOutput=True)

    sb = lambda name, shape, dt: nc.alloc_sbuf_tensor(name, shape, dt).ap()

    # planes: one alloc, free order (tensor o/t, comp x/y/h, px)
    mpl = sb("mpl", [128, 6 * P], F16)
    spl = sb("spl", [128, 6 * SP], F16)
    mpl4 = mpl.rearrange("p (t c px) -> p t c px", t=2, c=3, px=P)
    spl4 = spl.rearrange("p (t c px) -> p t c px", t=2, c=3, px=SP)

    def mk_mids(pre, n):
        m = {}
        for nm, width, dt, bufs in (
            ("iwih", 2 * n, F16, 1), ("cwch", 2 * n, F16, 1),
            ("pqgh", 2 * n, F16, 2), ("t12", 2 * n, F16, 2),
            ("d", n, F16, 2), ("inter", n, F16, 1), ("ac", n, F16, 2),
            ("s", n, F16, 1), ("ue", n, F16, 2),
            ("rcu32", n, F32, 2), ("rcc32", n, F32, 2),
            ("ascr", n, F16, 1), ("vscr", n, F16, 1),
        ):
            m[nm] = [sb(f"{pre}{nm}{i}", [128, width], dt) for i in range(bufs)]
        return m

    mm = mk_mids("m_", QP)
    sm = mk_mids("s_", SP)
    osb = sb("osb", [128, 24], F32)
    dmy = sb("dmy", [128, 4], F32)

    dma_side = nc.alloc_semaphore("dma_side")
    dma_q = [nc.alloc_semaphore(f"dma_q{q}") for q in range(len(CHUNKS))]
    dma_xy = [nc.alloc_semaphore(f"dma_xy{q}") for q in range(len(CHUNKS))]
    dma_out = nc.alloc_semaphore("dma_out")
    act_c = nc.alloc_semaphore("act_c")
    dve_c = nc.alloc_semaphore("dve_c")
    gp_c = nc.alloc_semaphore("gp_c")
    init_c = nc.alloc_semaphore("init_c")
    wt = _Waiter()

    # DRAM component-plane views: [b, k, c, px]
    o_v = o_ext.rearrange("b (k c) hx hy -> b k c (hx hy)", k=K, c=3)
    t_v = t_ext.rearrange("b (k c) hx hy -> b k c (hx hy)", k=K, c=3)

    # --- init + ACT table warmup (before any gating) ---
    nc.gpsimd.memset(dmy[:], 0.0).then_inc(init_c, 1)
    nc.gpsimd.memset(osb[:], 0.0).then_inc(init_c, 1)
    nc.vector.wait_ge(init_c, 2)
    nc.scalar.wait_ge(init_c, 2)
    nc.scalar.activation(dmy[:, 0:1], dmy[:, 3:4], AF.Abs)
    nc.scalar.activation(dmy[:, 1:2], dmy[:, 3:4], AF.Square)
    nc.scalar.add_instruction(
        mybir.InstActivation(
            name=nc.get_next_instruction_name(),
            func=AF.Reciprocal,
            ins=[nc.scalar.lower_ap(dmy[:, 3:4]),
                 mybir.ImmediateValue(dtype=F32, value=1.0),
                 mybir.ImmediateValue(dtype=F32, value=1.0),
                 mybir.ImmediateValue(dtype=F32, value=0.0)],
            outs=[nc.scalar.lower_ap(dmy[:, 2:3])],
        )
    )

    # --- DMA (gpsimd SWDGE, f32->f16 cast): geometric px-chunks, side last ---
    for ci2, (p0, pn) in enumerate(CHUNKS):
        # x/y planes first (own sem, gate 64); hm planes after (gate 32)
        for ti, ci in ((0, 1), (0, 2), (1, 1), (1, 2), (0, 0), (1, 0)):
            src = o_v if ti == 0 else t_v
            sem = dma_xy[ci2] if ci else dma_q[ci2]
            nc.gpsimd.dma_start(
                out=mpl4[:, ti, ci, p0:p0 + pn],
                in_=src[:, 0:KM, ci, p0:p0 + pn],
            ).then_inc(sem, 16)
    for ti, src in ((0, o_v), (1, t_v)):
        for ci in range(3):
            # sidecar: k=16, partition = b*16 + (px>>8)
            nc.gpsimd.dma_start(
                out=spl4[:, ti, ci],
                in_=src[:, KM, ci].rearrange("b (ph pl) -> b ph pl",
                                             ph=16, pl=SP),
            ).then_inc(dma_side, 16)

    def act_recip(out, in_, bias):
        return nc.scalar.add_instruction(
            mybir.InstActivation(
                name=nc.get_next_instruction_name(),
                func=AF.Reciprocal,
                ins=[nc.scalar.lower_ap(in_),
                     mybir.ImmediateValue(dtype=F32, value=bias),
                     mybir.ImmediateValue(dtype=F32, value=1.0),
                     mybir.ImmediateValue(dtype=F32, value=0.0)],
                outs=[nc.scalar.lower_ap(out)],
            )
        )

    def chain(step, pl4, n, m, col_q1, col_q2, col_d, gate_sem):
        """One chain-step (side=step0 on [128,SP]; quarter q=step q+1 on a
        QP slice). pl4: [p, t, c, px-window] plane view for this step."""
        d0, a0, g0 = ND * step, NA * step, NG * step
        dp, ap, gp = ND * (step - 2), NA * (step - 2), NG * (step - 2)
        par = step % 2

        def mb(nm):
            bufs = m[nm]
            t = bufs[par % len(bufs)]
            if nm in ("iwih", "cwch", "pqgh", "t12"):
                return t[:]
            return t[:, 0:n]

        o_xy = pl4[:, 0, 1:3]
        t_xy = pl4[:, 1, 1:3]
        x_ot = pl4[:, :, 1]
        y_ot = pl4[:, :, 2]
        o_h = pl4[:, 0, 0]
        t_h = pl4[:, 1, 0]
        iw2 = mb("iwih").rearrange("p (c px) -> p c px", c=2)[:, :, 0:n]
        cw2 = mb("cwch").rearrange("p (c px) -> p c px", c=2)[:, :, 0:n]

        x_ot = pl4[:, :, 1]
        y_ot = pl4[:, :, 2]
        t122 = mb("t12").rearrange("p (c px) -> p c px", c=2)[:, :, 0:n]
        pqw = mb("pqgh").rearrange("p (c px) -> p c px", c=2)[:, :, 0:n]

        # DVE (x/y planes arrive first: front gates on xy sem, d on hm sem)
        if m is mm:
            wt.wait(nc.vector, dma_xy[step], 64)
        else:
            wt.wait(nc.vector, gate_sem, 96)
        nc.vector._custom_dve(IW_OP, out=iw2, in0=o_xy, in1=t_xy).then_inc(dve_c, 1)
        nc.vector._custom_dve(CW_OP, out=cw2, in0=o_xy, in1=t_xy).then_inc(dve_c, 1)
        if step >= 2:
            wt.wait(nc.vector, act_c, ap + 1)       # pqgh read by t12(s-2)
        nc.vector.tensor_tensor(pqw, x_ot, y_ot, Alu.mult).then_inc(dve_c, 1)
        if step >= 2:
            wt.wait(nc.vector, act_c, ap + 4)       # d read by dsq(s-2)
        wt.wait(nc.vector, gate_sem, 32 if m is mm else 96)
        nc.vector.tensor_tensor(mb("d"), o_h, t_h, Alu.subtract).then_inc(dve_c, 1)
        nc.vector.tensor_tensor(mb("inter"), iw2[:, 0], iw2[:, 1], Alu.mult).then_inc(dve_c, 1)
        if step >= 2:
            wt.wait(nc.vector, act_c, ap + 2)       # ac read by rcc32(s-2)
        nc.vector.tensor_tensor(mb("ac"), cw2[:, 0], cw2[:, 1], Alu.mult).then_inc(dve_c, 1)
        wt.wait(nc.vector, act_c, a0 + 1)           # t12 ready
        nc.vector.tensor_tensor(mb("s"), t122[:, 0], t122[:, 1], Alu.add).then_inc(dve_c, 1)
        if step >= 2:
            wt.wait(nc.vector, act_c, ap + 3)       # ue read by rcu32(s-2)
        nc.vector.tensor_tensor(mb("ue"), mb("s"), mb("inter"), Alu.subtract).then_inc(dve_c, 1)
        wt.wait(nc.vector, act_c, a0 + 2)           # rcc32 ready
        nc.vector.scalar_tensor_tensor(
            mb("vscr"), mb("ue"), 1.0, mb("rcc32"), Alu.mult, Alu.mult,
            accum_out=col_q2).then_inc(dve_c, 1)
        wt.wait(nc.vector, act_c, a0 + 3)           # rcu32 ready
        nc.vector.scalar_tensor_tensor(
            mb("vscr"), mb("inter"), 1.0, mb("rcu32"), Alu.mult, Alu.mult,
            accum_out=col_q1).then_inc(dve_c, 1)

        # ACT
        wt.wait(nc.scalar, dve_c, d0 + 3)
        if step >= 2:
            wt.wait(nc.scalar, dve_c, dp + 7)       # t12 read by s(s-2)
        nc.scalar.activation(t122, pqw, AF.Abs).then_inc(act_c, 1)
        wt.wait(nc.scalar, dve_c, d0 + 6)
        if step >= 2:
            wt.wait(nc.scalar, dve_c, dp + 9)       # rcc32 read by q2(s-2)
        act_recip(mb("rcc32"), mb("ac"), EPS).then_inc(act_c, 1)
        wt.wait(nc.scalar, dve_c, d0 + 8)
        if step >= 2:
            wt.wait(nc.scalar, dve_c, dp + 10)      # rcu32 read by q1(s-2)
        act_recip(mb("rcu32"), mb("ue"), EPS).then_inc(act_c, 1)
        wt.wait(nc.scalar, dve_c, d0 + 4)
        nc.scalar.activation(mb("ascr"), mb("d"), AF.Square,
                             accum_out=col_d).then_inc(act_c, 1)

    # main chunks in order, sidecar last
    NC_ = len(CHUNKS)
    for ci2, (p0, pn) in enumerate(CHUNKS):
        chain(ci2, mpl4[:, :, :, p0:p0 + pn], pn, mm,
              osb[:, ci2:ci2 + 1], osb[:, NC_ + ci2:NC_ + ci2 + 1],
              osb[:, 2 * NC_ + ci2:2 * NC_ + ci2 + 1], dma_q[ci2])
    chain(NC_, spl4, SP, sm,
          osb[:, 3 * NC_:3 * NC_ + 1], osb[:, 3 * NC_ + 1:3 * NC_ + 2],
          osb[:, 3 * NC_ + 2:3 * NC_ + 3], dma_side)

    # epilogue
    wt.wait(nc.sync, dve_c, ND * (len(CHUNKS) + 1))
    wt.wait(nc.sync, act_c, NA * (len(CHUNKS) + 1))
    nc.sync.dma_start(out=p_ext[:], in_=osb[:]).then_inc(dma_out, 16)
    nc.sync.wait_ge(dma_out, 16)

    mybir.codegen_inst_isa_subclasses(nc)
    return nc


_NC = None


def _get_nc():
    global _NC
    if _NC is None:
        _NC = build_nc()
    return _NC


def _combine(parts, target_weights):
    """parts: [8 cores, 128, 16] f32 -> scalar loss (host-side finish)."""
    arr = np.asarray(parts, np.float64).reshape(N_CORES, B_LOC, 16, 24)
    nch = len(CHUNKS)
    sqs = np.zeros((B, K))
    ssd = np.zeros((B, K))
    for i in range(N_CORES):
        for b in range(B_LOC):
            gb = i * B_LOC + b
            rows = arr[i, b]                      # [16 partitions, 24 cols]
            sqs[gb, :KM] = rows[:, 0:2 * nch].sum(1)
            ssd[gb, :KM] = rows[:, 2 * nch:3 * nch].sum(1)
            sqs[gb, KM] = rows[:, 3 * nch].sum() + rows[:, 3 * nch + 1].sum()
            ssd[gb, KM] = rows[:, 3 * nch + 2].sum()

    tw = np.asarray(target_weights, np.float64)
    twnz = (tw != 0).astype(np.float64)
    num = ((2.0 * P - sqs) * twnz).sum(axis=0)
    den = np.maximum((P * twnz).sum(axis=0), 1.0)
    giou_joint = num / den
    mse = 0.5 * (tw**2 * ssd).sum(axis=0) / (B * P)
    return np.float32(np.sum(mse + giou_joint) / K)


def kernel(output, target, target_weights):
    output = np.ascontiguousarray(np.asarray(output), dtype=np.float32)
    target = np.ascontiguousarray(np.asarray(target), dtype=np.float32)
    nc = _get_nc()
    in_maps = [
        {
            "output": output[i * B_LOC:(i + 1) * B_LOC],
            "target": target[i * B_LOC:(i + 1) * B_LOC],
        }
        for i in range(N_CORES)
    ]
    res = run_bass_kernel_spmd(nc, in_maps, list(range(N_CORES)))
    parts = np.stack([res.results[i]["partials"] for i in range(N_CORES)])
    return np.asarray(_combine(parts, target_weights), dtype=np.float32)
